# revision 50
# baseline (speedup 1.0000x reference)
"""Trainium2 Bass kernel for nn_AttentionLayer (linear attention, conv1x1 projections).

Math (per batch b, with x flattened to [C=512, L=4096]):
    QP = Wq @ x + bq ; Q = elu(QP)+1
    KP = Wk @ x + bk ; K = elu(KP)+1
    VP = Wv @ x + bv          (reference divides by L here and multiplies by L
                               at the end -- exact cancellation, so we drop both)
    per head h (64 channels each):
        KV_h   = K_h @ V_h^T                  [64, 64]
        Ksum_h = K_h @ ones                   [64]
        S_h[l] = Ksum_h . Q_h[:, l]
        out_h  = (KV_h^T @ Q_h) / S_h         (eps=1e-6 negligible vs S~1e5)
    y = Wo @ out + bo

Distribution: data-parallel over batch, 1 batch per NeuronCore (8 cores).
All matmuls run in bf16 (f32 PSUM accumulate).  elu(x)+1 is computed exactly as
min(exp(x), 1) + relu(x)  (uses exp(x) >= x+1, equality at 0).

Layouts on chip:
    Q   [c, l]  (normal)      -- rhs for S/out matmuls, lhsT = per-head blocks
    K^T [l, c]  (transposed)  -- produced directly by using x-chunks as lhsT
    V^T [l, c]  (transposed)
    KV_bd  [c-chunk, 128]: per 128-chunk m, block-diag(KV_2m, KV_2m+1)
    KsumRep[c-chunk, 128]: block-diag(Ksum_2m 1^T, Ksum_2m+1 1^T) -> S replicated
        to all 128 partitions so the reciprocal multiply needs no partition
        broadcast.
"""

import sys

import numpy as np

if "/opt/trn_rl_repo" not in sys.path:
    sys.path.insert(0, "/opt/trn_rl_repo")

import ml_dtypes

BF16 = ml_dtypes.bfloat16

_RECIP1_MUL_CONSTS = (-0.23549794, 2.00173235)


def _register_recip1_mul():
    """Register a fused out = in1 * approx_recip(in0) custom-DVE op.

    Same BITWISE_NOT exponent-flip seed as reciprocal_approx_fast but a
    single Newton-Raphson pass (~0.17% max rel err, fine vs the 2e-2 gate)
    with the Z-scale multiply fused in -- one full-rate DVE pass instead of
    the recip+mul pair, which is what makes phase 2 PE-bound.
    """
    from concourse import dve_ops
    from concourse.dve_spec import AluOp, Bin, C0, C1, Spec, Src0, Src1

    for op in dve_ops.OPS:
        if op.name == "RECIP1_MUL_ANT":
            return op
    _not = Bin(AluOp.BITWISE_NOT, Src0, Src0)
    _y0 = _not * C0

    def _ref(in0, in1, s0, s1, imm2):
        nb = (~np.asarray(in0, np.float32).view(np.int32)).view(np.float32)
        y0 = nb * s0
        return in1 * (y0 * (s1 - in0 * y0))

    op = dve_ops.DveOp(
        "RECIP1_MUL_ANT",
        Spec(body=Src1 * (_y0 * (C1 - Src0 * _y0)), reference=_ref),
        subdim=False,
        uops_sha={"v3": "819e5f132feeb6b1", "v4": "86bbdf11bfdf9f38"},
    )
    dve_ops.OPS.append(op)
    dve_ops.CUSTOM_DVE_SPECS[op.name] = op.spec
    dve_ops._SUB_OPCODE_FOR_NAME[op.name] = (
        dve_ops._CUSTOM_DVE_ROW_BASE + len(dve_ops.OPS) - 1)
    return op

C = 512
L = 4096
NB = 8          # batches == cores
NCC = 4         # 128-row chunks of C
NL512 = 8       # 512-col chunks of L
NL128 = 32      # 128-col chunks of L

_CACHE = {}


def _build_nc(debug_outputs=False):
    import concourse.bass as bass  # noqa: F401
    import concourse.tile as tile
    from concourse import bacc, mybir

    recip1_mul = _register_recip1_mul()

    f32 = mybir.dt.float32
    bf16 = mybir.dt.bfloat16
    AF = mybir.ActivationFunctionType
    OP = mybir.AluOpType

    nc = bacc.Bacc("TRN2", target_bir_lowering=False, debug=False,
                   enable_asserts=False, num_devices=NB)

    dbg = {}
    if debug_outputs:
        dbg["Q"] = nc.dram_tensor("dbg_Q", [128, NCC, L], bf16, kind="ExternalOutput")
        dbg["Kt"] = nc.dram_tensor("dbg_Kt", [128, NL128, C], bf16, kind="ExternalOutput")
        dbg["Vt"] = nc.dram_tensor("dbg_Vt", [128, NL128, NCC, 129], bf16, kind="ExternalOutput")
        dbg["KVbd"] = nc.dram_tensor("dbg_KVbd", [128, NCC, 128], bf16, kind="ExternalOutput")
        dbg["KsumRep"] = nc.dram_tensor("dbg_KsumRep", [128, NCC, 128], bf16, kind="ExternalOutput")
        dbg["Om"] = nc.dram_tensor("dbg_Om", [128, NCC, L], bf16, kind="ExternalOutput")

    x_d = nc.dram_tensor("x", [C, L], bf16, kind="ExternalInput")
    wq_d = nc.dram_tensor("wqT", [C, C], bf16, kind="ExternalInput")
    wk_d = nc.dram_tensor("wkT", [C, C], bf16, kind="ExternalInput")
    wv_d = nc.dram_tensor("wvT", [C, C], bf16, kind="ExternalInput")
    wo_d = nc.dram_tensor("woT", [C, C], bf16, kind="ExternalInput")
    bq_d = nc.dram_tensor("bqT", [128, NCC], f32, kind="ExternalInput")
    bo_d = nc.dram_tensor("boT", [128, NCC], f32, kind="ExternalInput")
    bkb_d = nc.dram_tensor("bkb", [128, C], f32, kind="ExternalInput")
    bvb_d = nc.dram_tensor("bvb", [128, C], f32, kind="ExternalInput")
    eye_d = nc.dram_tensor("eye", [128, 128], bf16, kind="ExternalInput")
    out_d = nc.dram_tensor("out", [C, L], f32, kind="ExternalOutput")

    x_ap = x_d.ap().rearrange("(cc p) l -> p cc l", p=128)   # [128, 4, L]
    out_ap = out_d.ap()

    from contextlib import ExitStack

    with tile.TileContext(nc) as tc:
        with ExitStack() as stack:
            const = stack.enter_context(tc.tile_pool(name="const", bufs=1))
            big = stack.enter_context(tc.tile_pool(name="big", bufs=1))
            xin = stack.enter_context(tc.tile_pool(name="xin", bufs=3))
            ev = stack.enter_context(tc.tile_pool(name="ev", bufs=3))
            # ---- constants (load order = first-use order) ----
            wq_sb = const.tile([128, NCC, C], bf16)
            wk_sb = const.tile([128, NCC, C], bf16)
            wv_sb = const.tile([128, NCC, C], bf16)
            wo_sb = const.tile([128, NCC, C], bf16)
            bq_sb = const.tile([128, NCC], f32)
            bo_sb = const.tile([128, NCC], f32)
            bkb_sb = const.tile([128, C], f32)
            bvb_sb = const.tile([128, C], f32)
            # Startup loads split across both HWDGE queues (SP + ACT): the
            # K-projection path (x j=0 interleaved with wk, then bkb) on SP,
            # everything else on the ACT queue.  The first kp matmul only
            # needs (x cc=0, wk cc=0), so interleaving starts PE ~1.3us in.
            # Startup loads run on two queues in parallel: x chunks on SP,
            # every weight on the otherwise-idle GpSimd (Pool) queue.  The
            # ACT queue stays free for the activation stream from t=0.
            # j=0 runs both K/V halves before any Q work, so wq only needs
            # to land ~7us in.
            xt0 = xin.tile([128, NCC, 512], bf16, name="xt0", tag="xt")
            for cc in range(NCC):
                nc.sync.dma_start(out=xt0[:, cc, :], in_=x_ap[:, cc, 0:512])
                nc.gpsimd.dma_start(out=wk_sb[:, cc, :],
                                    in_=wk_d.ap()[128 * cc:128 * (cc + 1), :])
            nc.sync.dma_start(out=bkb_sb, in_=bkb_d.ap())
            for cc in range(NCC):
                nc.gpsimd.dma_start(out=wv_sb[:, cc, :],
                                    in_=wv_d.ap()[128 * cc:128 * (cc + 1), :])
            nc.gpsimd.dma_start(out=bvb_sb, in_=bvb_d.ap())
            nc.scalar.dma_start(out=bq_sb, in_=bq_d.ap())
            for cc in range(NCC):
                nc.scalar.dma_start(out=wq_sb[:, cc, :],
                                    in_=wq_d.ap()[128 * cc:128 * (cc + 1), :])
            for cc in range(NCC):
                nc.gpsimd.dma_start(out=wo_sb[:, cc, :],
                                    in_=wo_d.ap()[128 * cc:128 * (cc + 1), :])
            nc.gpsimd.dma_start(out=bo_sb, in_=bo_d.ap())
            ones128_sb = const.tile([128, 64], bf16)
            nc.vector.memset(ones128_sb, 1.0)
            eye_sb = const.tile([128, 128], bf16)
            nc.gpsimd.dma_start(out=eye_sb, in_=eye_d.ap())

            # ---- persistent activations ----
            Q_sb = big.tile([128, NCC, L], bf16)     # [c, l] normal
            Kt_sb = big.tile([128, NL128, C], bf16)  # [l, c] transposed
            # V^T stored per m-chunk with a trailing ones column: [l, m, 129]
            # so the KV matmul's 129th output column accumulates Ksum in the
            # SAME psum chain (start=True zeroes a whole 2KB bank, so each
            # chain needs a private bank and no sibling chains).
            Vt_sb = big.tile([128, NL128, NCC, 129], bf16)
            nc.vector.memset(Vt_sb[:, :, :, 128:129], 1.0)
            # W~ = Wo @ blockdiag(KV_h^T) folded at the phase boundary: the
            # attention-out matmul then fuses into the O-projection (y =
            # W~ @ (Q*Z) + bo), dropping 32 out-matmuls from phase 2.
            KVbdT_sb = const.tile([128, NCC, 128], bf16)
            WtT_sb = const.tile([128, NCC, C], bf16)
            KVbd_sb = const.tile([128, NCC, 128], bf16)
            KsumRep_sb = const.tile([128, NCC, 128], bf16)
            ksum_sb = const.tile([128, NCC], f32)
            # zero the off-diagonal blocks up front (not data dependent)
            for m in range(NCC):
                nc.vector.memset(KVbd_sb[0:64, m, 64:128], 0.0)
                nc.vector.memset(KVbd_sb[64:128, m, 0:64], 0.0)
                nc.vector.memset(KsumRep_sb[0:64, m, 64:128], 0.0)
                nc.vector.memset(KsumRep_sb[64:128, m, 0:64], 0.0)

            def bcast_pair(ap):
                """View an AP ([128, ...]) with a broadcast pair dim inserted
                after partitions: [128, 2(step 0), ...]."""
                return bass.AP(tensor=ap.tensor, offset=ap.offset,
                               ap=[list(ap.ap[0]), [0, 2],
                                   *[list(d) for d in ap.ap[1:]]])

            # ================= phase 1: projections + KV accumulation =========
            with ExitStack() as p1stack:
                # PSUM (8 banks): kp x2 + vp x2 (4) + qp pair (2)
                #               + KV accumulators paired 2-per-bank (2)
                pkv = p1stack.enter_context(tc.tile_pool(name="pkv", bufs=2, space="PSUM"))
                pq = p1stack.enter_context(tc.tile_pool(name="pq", bufs=1, space="PSUM"))
                pacc = p1stack.enter_context(tc.tile_pool(name="pacc", bufs=1, space="PSUM"))
                # Two chains genuinely share one bank (chain g at 1KB offset
                # g*256 f32): only the temporally-first matmul in a bank
                # carries start=True -- start clears the has_written bits of
                # the WHOLE 2KB bank, so the second chain's first matmul must
                # NOT carry start or it would wipe its bank-sibling's li=0
                # contribution.  [128,129] at offsets 0/1024/2048/3072 never
                # crosses a bank boundary, keeping each matmul single-bank.
                KV_all = pacc.tile([128, 4, 256], f32, tag="kvacc", name="kv_all")
                KV_ps = [KV_all[:, g, 0:129] for g in range(4)]

                bvb4 = bvb_sb[:, :].rearrange("p (m c) -> p m c", m=NCC)
                bkb2 = bcast_pair(bkb_sb[:, :])
                bvb4x2 = bcast_pair(bvb4)
                # Q's final combine is deferred one half-iteration: it is only
                # consumed in phase 2, and emitting it late keeps the DVE FIFO
                # clear so the next half's psum-releasing adds run sooner.
                pending_qstt = []
                # KV-accumulation matmuls are likewise deferred one half:
                # they depend on the elementwise K/V chain (~4-5us deep) and
                # the PE instruction FIFO is strict in-order, so emitting them
                # immediately would head-of-line-block the next half's
                # independent projection matmuls.
                pending_kv = []

                def emit_kv(li0):
                    for jj2 in range(2):
                        li = li0 + jj2
                        for m in range(NCC):
                            ks = Kt_sb[:, li, 128 * m:128 * (m + 1)]
                            nc.tensor.matmul(
                                KV_ps[m], ks, Vt_sb[:, li, m, :],
                                start=(li == 0 and m % 2 == 0),
                                stop=(li == NL128 - 1))
                for j in range(NL512):
                    if j == 0:
                        xt = xt0
                    else:
                        xt = xin.tile([128, NCC, 512], bf16, name="xt", tag="xt")
                        nc.sync.dma_start(out=xt,
                                          in_=x_ap[:, :, 512 * j:512 * (j + 1)])

                    def kv_half(j, half, xt):
                        # Both 128-l chunks of this half processed as one
                        # [128, 2, 512] pair: halves the per-op fixed cost on
                        # every DVE/ACT instruction in the K/V chain.
                        li0 = 4 * j + 2 * half
                        kp = pkv.tile([128, 2, 512], f32, tag="kp", name="kp", bufs=1)
                        vp = pkv.tile([128, 2, 512], f32, tag="vp", name="vp", bufs=1)
                        for jj2 in range(2):
                            lf = 128 * (2 * half + jj2)
                            for cc in range(NCC):
                                xs = xt[:, cc, lf:lf + 128]
                                nc.tensor.matmul(kp[:, jj2, :], xs, wk_sb[:, cc, :],
                                                 start=(cc == 0), stop=(cc == NCC - 1))
                        for jj2 in range(2):
                            lf = 128 * (2 * half + jj2)
                            for cc in range(NCC):
                                xs = xt[:, cc, lf:lf + 128]
                                nc.tensor.matmul(vp[:, jj2, :], xs, wv_sb[:, cc, :],
                                                 start=(cc == 0), stop=(cc == NCC - 1))
                        # K^T = elu(kp + bk) + 1 = min(exp(t),1) + relu(t)
                        t0 = ev.tile([128, 2, 512], f32, tag="t0", bufs=3, name="t0")
                        e0 = ev.tile([128, 2, 512], bf16, tag="e0", bufs=4, name="e0")
                        r0 = ev.tile([128, 2, 512], bf16, tag="r0", bufs=4, name="r0")
                        nc.vector.tensor_add(t0, kp, bkb2)
                        # V^T = vp + bv  (written into the 129-strided layout)
                        nc.vector.tensor_add(
                            Vt_sb[:, li0:li0 + 2, :, 0:128],
                            vp.rearrange("p two (m c) -> p two m c", m=NCC),
                            bvb4x2)
                        nc.scalar.activation(e0, t0, AF.Exp)
                        nc.scalar.activation(r0, t0, AF.Relu)
                        nc.vector.scalar_tensor_tensor(
                            Kt_sb[:, li0:li0 + 2, :], e0, 1.0, r0, OP.min, OP.add)
                        pending_kv.append(li0)

                    def q_proj(j, op2, xt):
                        # Q projection, one oi-pair (spreads ACT load).  Two
                        # single-bank psum tags rotate so oi+1's matmuls can
                        # run while oi's ACT evictions still read their bank.
                        eq = ev.tile([128, 2, 512], bf16, tag="e0", bufs=4, name="eq")
                        rq = ev.tile([128, 2, 512], bf16, tag="r0", bufs=4, name="rq")
                        for oi2 in range(2):
                            oi = 2 * op2 + oi2
                            qp = pq.tile([128, 512], f32, tag=f"qp{oi2}",
                                         name=f"qp{oi2}")
                            for cc in range(NCC):
                                nc.tensor.matmul(
                                    qp,
                                    wq_sb[:, cc, 128 * oi:128 * (oi + 1)],
                                    xt[:, cc, :],
                                    start=(cc == 0), stop=(cc == NCC - 1))
                            nc.scalar.activation(eq[:, oi2, :], qp,
                                                 AF.Exp, bias=bq_sb[:, oi:oi + 1])
                            # relu+bias on DVE: the qp bank frees after ONE
                            # ACT pass + one parallel DVE pass, so the next
                            # oi's matmuls never wait on a second ACT op
                            nc.vector.tensor_scalar(
                                rq[:, oi2, :], qp, bq_sb[:, oi:oi + 1], 0.0,
                                OP.add, OP.max)
                            if pending_qstt:
                                nc.vector.scalar_tensor_tensor(*pending_qstt.pop())
                            pending_qstt.append(
                                (Q_sb[:, oi, 512 * j:512 * (j + 1)],
                                 eq[:, oi2, :], 1.0, rq[:, oi2, :],
                                 OP.min, OP.add))

                    if j == 0:
                        # first chunk: both K/V halves before any Q work, so
                        # the PE never waits on the later-arriving wq and the
                        # ACT queue warms up on the K-path exp/relu stream
                        kv_half(j, 0, xt)
                        kv_half(j, 1, xt)
                        emit_kv(pending_kv.pop(0))
                        q_proj(j, 0, xt)
                        q_proj(j, 1, xt)
                    elif j < NL512 - 1:
                        for half in range(2):
                            kv_half(j, half, xt)
                            if len(pending_kv) > 1:
                                emit_kv(pending_kv.pop(0))
                            q_proj(j, half, xt)
                    else:
                        # last chunk: K/V first so the final elementwise chain
                        # overlaps the Q projections, keeping the dangling
                        # stt -> KV-matmul tail at the phase boundary short
                        kv_half(j, 0, xt)
                        emit_kv(pending_kv.pop(0))
                        kv_half(j, 1, xt)
                        q_proj(j, 0, xt)
                        emit_kv(pending_kv.pop(0))
                        q_proj(j, 1, xt)
                while pending_kv:
                    emit_kv(pending_kv.pop(0))

                # ---- evict Ksum + KsumRep on DVE (S(j=0) depends only on
                # these); the KVbd blocks evict on ACT (idle at the
                # boundary) so both chains run in parallel ----
                for m in range(NCC):
                    nc.vector.tensor_copy(ksum_sb[:, m:m + 1],
                                          KV_ps[m][:, 128:129])
                for m in range(NCC):
                    nc.vector.tensor_scalar_mul(
                        KsumRep_sb[0:64, m, 0:64], ones128_sb[0:64, :],
                        ksum_sb[0:64, m:m + 1])
                    nc.vector.tensor_scalar_mul(
                        KsumRep_sb[64:128, m, 64:128], ones128_sb[64:128, :],
                        ksum_sb[64:128, m:m + 1])
                for m in range(NCC):
                    kv_m = KV_ps[m]
                    nc.scalar.copy(KVbd_sb[0:64, m, 0:64], kv_m[0:64, 0:64])
                    nc.scalar.copy(KVbd_sb[64:128, m, 64:128],
                                   kv_m[64:128, 64:128])
                # the last Q combine is only consumed by phase-2 j=7 -- flush
                # it AFTER the KV eviction chain so it doesn't delay S_b(j=0)
                if pending_qstt:
                    nc.vector.scalar_tensor_tensor(*pending_qstt.pop())

            if debug_outputs:
                nc.sync.dma_start(out=dbg["Q"].ap(), in_=Q_sb)
                nc.sync.dma_start(out=dbg["Kt"].ap(), in_=Kt_sb)
                nc.sync.dma_start(out=dbg["Vt"].ap(), in_=Vt_sb)
                nc.sync.dma_start(out=dbg["KVbd"].ap(), in_=KVbd_sb)
                nc.sync.dma_start(out=dbg["KsumRep"].ap(), in_=KsumRep_sb)

            # ================= phase 2: fold + O-projection ===================
            # W~T[m] = KVbd[m]^T @ WoT[m] is built once at the boundary; the
            # j-loop then only needs S (KsumRep matmuls) -> 1/S -> scale Q in
            # place -> y = W~T^T @ Q~.  Z rides on Q, so the old per-j
            # attention-out matmuls disappear entirely.
            with ExitStack() as p2stack:
                # PSUM banks: sb pair (2) + y pair x2 bufs (4) + tp (1);
                # the wt fold tiles share the y0 tag's banks.
                p2 = p2stack.enter_context(tc.tile_pool(name="p2", bufs=1, space="PSUM"))
                p2o = p2stack.enter_context(tc.tile_pool(name="p2o", bufs=2, space="PSUM"))
                # ytp deep enough that output-DMA latency never back-
                # pressures the ACT evictions (and with them the y matmuls)
                ytp = p2stack.enter_context(tc.tile_pool(name="ytp", bufs=4))

                def s_group(j):
                    lsl = slice(512 * j, 512 * (j + 1))
                    for mp in range(2):
                        sbp = p2.tile([128, 2, 512], f32, tag="sb", name="sbp",
                                      bufs=2)
                        for m2 in range(2):
                            m = 2 * mp + m2
                            nc.tensor.matmul(sbp[:, m2, :], KsumRep_sb[:, m, :],
                                             Q_sb[:, m, lsl], start=True, stop=True)
                        # Q~ = Q/S in place via the fused approx-recip+mul op
                        # (S above already read this slice)
                        qsl = Q_sb[:, 2 * mp:2 * mp + 2, lsl]
                        nc.vector._custom_dve(
                            recip1_mul, out=qsl, in0=sbp, in1=qsl,
                            s0=_RECIP1_MUL_CONSTS[0], s1=_RECIP1_MUL_CONSTS[1],
                            imm2=0.0)

                def fold_wt():
                    # Per-m pipelined fold: transpose chunk m via eye, copy
                    # it out, multiply by WoT[m], evict -- so W~T[m=0] (the
                    # first lhsT the y matmuls need) is ready ~2us after the
                    # KVbd eviction instead of after the whole fold.  tp
                    # borrows an "sb" slot (only live across the boundary).
                    tp = p2.tile([128, NCC, 128], f32, tag="sb", name="tp",
                                 bufs=2)
                    for m in range(NCC):
                        nc.tensor.matmul(tp[:, m, :], KVbd_sb[:, m, :], eye_sb,
                                         start=(m == 0), stop=(m == NCC - 1))
                        nc.scalar.copy(KVbdT_sb[:, m, :], tp[:, m, :])
                        wt = p2o.tile([128, 512], f32, tag="y0", name="wt", bufs=2)
                        nc.tensor.matmul(wt, KVbdT_sb[:, m, :], wo_sb[:, m, :],
                                         start=True, stop=True)
                        if m % 2 == 0:
                            nc.scalar.activation(WtT_sb[:, m, :], wt, AF.Identity)
                        else:
                            nc.vector.tensor_copy(WtT_sb[:, m, :], wt)

                # output DMAs rotate across four queues so the write-back
                # never serializes behind one HWDGE queue; fine mode drains
                # per-oi as soon as each chunk is evicted.
                dma_qs = [nc.sync, nc.gpsimd, nc.scalar, nc.gpsimd]
                def y_block(j, fine=False):
                    lsl = slice(512 * j, 512 * (j + 1))
                    for op2 in range(2):
                        dma_q = dma_qs[(2 * j + op2) % len(dma_qs)]
                        yt = ytp.tile([128, 2, 512], f32, name="yt")
                        for oi2 in range(2):
                            oi = 2 * op2 + oi2
                            yp = p2o.tile([128, 512], f32, tag=f"y{oi2}",
                                          name="yp", bufs=2)
                            for vi in range(NCC):
                                nc.tensor.matmul(
                                    yp,
                                    WtT_sb[:, vi, 128 * oi:128 * (oi + 1)],
                                    Q_sb[:, vi, lsl],
                                    start=(vi == 0), stop=(vi == NCC - 1))
                            nc.scalar.activation(yt[:, oi2, :], yp,
                                                 AF.Identity, bias=bo_sb[:, oi:oi + 1])
                            if fine:  # drain the tail with per-oi DMAs
                                dma_qs[oi % 3].dma_start(
                                    out=out_ap[128 * oi:128 * (oi + 1), lsl],
                                    in_=yt[:, oi2, :])
                        if not fine:
                            dma_q.dma_start(
                                out=out_ap[256 * op2:256 * (op2 + 1), lsl].rearrange(
                                    "(two p) l -> p two l", p=128),
                                in_=yt)

                # Emission order at the boundary untangles the per-engine
                # FIFOs: tp needs only the KVbd eviction, s_group(0) needs
                # only KsumRep, and putting s_group(0)'s DVE work after the
                # KVbdT copy keeps fold_wt's matmuls from waiting behind the
                # j=0 recip chain.  y-blocks run one j behind so their
                # matmuls fill the PE while the next j's S/scale chain is in
                # flight.
                s_group(0)
                fold_wt()
                for j in range(1, NL512):
                    s_group(j)
                    y_block(j - 1, fine=(j == NL512 - 1))
                y_block(NL512 - 1, fine=True)

    nc.compile()
    return nc


def _get_nc():
    if "nc" not in _CACHE:
        _CACHE["nc"] = _build_nc()
    return _CACHE["nc"]


def _make_in_maps(inputs):
    x = np.asarray(inputs["x"], dtype=np.float32)
    wq = np.asarray(inputs["wq"], dtype=np.float32)
    wk = np.asarray(inputs["wk"], dtype=np.float32)
    wv = np.asarray(inputs["wv"], dtype=np.float32)
    wo = np.asarray(inputs["wo"], dtype=np.float32)
    bq = np.asarray(inputs["bq"], dtype=np.float32)
    bk = np.asarray(inputs["bk"], dtype=np.float32)
    bv = np.asarray(inputs["bv"], dtype=np.float32)
    bo = np.asarray(inputs["bo"], dtype=np.float32)

    shared = {
        "wqT": np.ascontiguousarray(wq.T).astype(BF16),
        "wkT": np.ascontiguousarray(wk.T).astype(BF16),
        "wvT": np.ascontiguousarray(wv.T).astype(BF16),
        "woT": np.ascontiguousarray(wo.T).astype(BF16),
        "bqT": np.ascontiguousarray(bq.reshape(NCC, 128).T),
        "boT": np.ascontiguousarray(bo.reshape(NCC, 128).T),
        "bkb": np.ascontiguousarray(np.broadcast_to(bk, (128, C))),
        "bvb": np.ascontiguousarray(np.broadcast_to(bv, (128, C))),
        "eye": np.eye(128, dtype=np.float32).astype(BF16),
    }
    in_maps = []
    for b in range(NB):
        m = dict(shared)
        m["x"] = np.ascontiguousarray(x[b].reshape(C, L)).astype(BF16)
        in_maps.append(m)
    return in_maps


def _run(inputs, trace=False):
    from concourse.bass_utils import run_bass_kernel_spmd

    nc = _get_nc()
    in_maps = _make_in_maps(inputs)
    res = run_bass_kernel_spmd(nc, in_maps, core_ids=list(range(NB)), trace=trace)
    outs = np.stack([np.asarray(res.results[b]["out"], dtype=np.float32)
                     for b in range(NB)])
    y = outs.reshape(NB, C, 64, 64)
    return y, res


def kernel(**inputs) -> np.ndarray:
    y, _ = _run(inputs, trace=False)
    return y



# revision 51
# speedup vs baseline: 1.0679x; 1.0679x over previous
"""Trainium2 Bass kernel for nn_AttentionLayer (linear attention, conv1x1 projections).

Math (per batch b, with x flattened to [C=512, L=4096]):
    QP = Wq @ x + bq ; Q = elu(QP)+1
    KP = Wk @ x + bk ; K = elu(KP)+1
    VP = Wv @ x + bv          (reference divides by L here and multiplies by L
                               at the end -- exact cancellation, so we drop both)
    per head h (64 channels each):
        KV_h   = K_h @ V_h^T                  [64, 64]
        Ksum_h = K_h @ ones                   [64]
        S_h[l] = Ksum_h . Q_h[:, l]
        out_h  = (KV_h^T @ Q_h) / S_h         (eps=1e-6 negligible vs S~1e5)
    y = Wo @ out + bo

Distribution: data-parallel over batch, 1 batch per NeuronCore (8 cores).
All matmuls run in bf16 (f32 PSUM accumulate).  elu(x)+1 is computed exactly as
min(exp(x), 1) + relu(x)  (uses exp(x) >= x+1, equality at 0).

Layouts on chip:
    Q   [c, l]  (normal)      -- rhs for S/out matmuls, lhsT = per-head blocks
    K^T [l, c]  (transposed)  -- produced directly by using x-chunks as lhsT
    V^T [l, c]  (transposed)
    KV_bd  [c-chunk, 128]: per 128-chunk m, block-diag(KV_2m, KV_2m+1)
    KsumRep[c-chunk, 128]: block-diag(Ksum_2m 1^T, Ksum_2m+1 1^T) -> S replicated
        to all 128 partitions so the reciprocal multiply needs no partition
        broadcast.
"""

import sys

import numpy as np

if "/opt/trn_rl_repo" not in sys.path:
    sys.path.insert(0, "/opt/trn_rl_repo")

import ml_dtypes

BF16 = ml_dtypes.bfloat16

_RECIP1_MUL_CONSTS = (-0.23549794, 2.00173235)


def _register_recip1_mul():
    """Register a fused out = in1 * approx_recip(in0) custom-DVE op.

    Same BITWISE_NOT exponent-flip seed as reciprocal_approx_fast but a
    single Newton-Raphson pass (~0.17% max rel err, fine vs the 2e-2 gate)
    with the Z-scale multiply fused in -- one full-rate DVE pass instead of
    the recip+mul pair, which is what makes phase 2 PE-bound.
    """
    from concourse import dve_ops
    from concourse.dve_spec import AluOp, Bin, C0, C1, Spec, Src0, Src1

    for op in dve_ops.OPS:
        if op.name == "RECIP1_MUL_ANT":
            return op
    _not = Bin(AluOp.BITWISE_NOT, Src0, Src0)
    _y0 = _not * C0

    def _ref(in0, in1, s0, s1, imm2):
        nb = (~np.asarray(in0, np.float32).view(np.int32)).view(np.float32)
        y0 = nb * s0
        return in1 * (y0 * (s1 - in0 * y0))

    op = dve_ops.DveOp(
        "RECIP1_MUL_ANT",
        Spec(body=Src1 * (_y0 * (C1 - Src0 * _y0)), reference=_ref),
        subdim=False,
        uops_sha={"v3": "819e5f132feeb6b1", "v4": "86bbdf11bfdf9f38"},
    )
    dve_ops.OPS.append(op)
    dve_ops.CUSTOM_DVE_SPECS[op.name] = op.spec
    dve_ops._SUB_OPCODE_FOR_NAME[op.name] = (
        dve_ops._CUSTOM_DVE_ROW_BASE + len(dve_ops.OPS) - 1)
    return op

C = 512
L = 4096
NB = 8          # batches == cores
NCC = 4         # 128-row chunks of C
NL512 = 8       # 512-col chunks of L
NL128 = 32      # 128-col chunks of L

_CACHE = {}


def _build_nc(debug_outputs=False):
    import concourse.bass as bass  # noqa: F401
    import concourse.tile as tile
    from concourse import bacc, mybir

    recip1_mul = _register_recip1_mul()

    f32 = mybir.dt.float32
    bf16 = mybir.dt.bfloat16
    AF = mybir.ActivationFunctionType
    OP = mybir.AluOpType

    nc = bacc.Bacc("TRN2", target_bir_lowering=False, debug=False,
                   enable_asserts=False, num_devices=NB)

    dbg = {}
    if debug_outputs:
        dbg["Q"] = nc.dram_tensor("dbg_Q", [128, NCC, L], bf16, kind="ExternalOutput")
        dbg["Kt"] = nc.dram_tensor("dbg_Kt", [128, NL128, C], bf16, kind="ExternalOutput")
        dbg["Vt"] = nc.dram_tensor("dbg_Vt", [128, NL128, NCC, 129], bf16, kind="ExternalOutput")
        dbg["KVbd"] = nc.dram_tensor("dbg_KVbd", [128, NCC, 128], bf16, kind="ExternalOutput")
        dbg["KsumRep"] = nc.dram_tensor("dbg_KsumRep", [128, NCC, 128], bf16, kind="ExternalOutput")
        dbg["Om"] = nc.dram_tensor("dbg_Om", [128, NCC, L], bf16, kind="ExternalOutput")

    x_d = nc.dram_tensor("x", [C, L], bf16, kind="ExternalInput")
    wq_d = nc.dram_tensor("wqT", [C, C], bf16, kind="ExternalInput")
    wk_d = nc.dram_tensor("wkT", [C, C], bf16, kind="ExternalInput")
    wv_d = nc.dram_tensor("wvT", [C, C], bf16, kind="ExternalInput")
    wo_d = nc.dram_tensor("woT", [C, C], bf16, kind="ExternalInput")
    bq_d = nc.dram_tensor("bqT", [128, NCC], f32, kind="ExternalInput")
    bo_d = nc.dram_tensor("boT", [128, NCC], f32, kind="ExternalInput")
    bkb_d = nc.dram_tensor("bkb", [128, C], f32, kind="ExternalInput")
    bvb_d = nc.dram_tensor("bvb", [128, C], f32, kind="ExternalInput")
    eye_d = nc.dram_tensor("eye", [128, 128], bf16, kind="ExternalInput")
    out_d = nc.dram_tensor("out", [C, L], f32, kind="ExternalOutput")

    x_ap = x_d.ap().rearrange("(cc p) l -> p cc l", p=128)   # [128, 4, L]
    out_ap = out_d.ap()

    from contextlib import ExitStack

    with tile.TileContext(nc) as tc:
        with ExitStack() as stack:
            const = stack.enter_context(tc.tile_pool(name="const", bufs=1))
            big = stack.enter_context(tc.tile_pool(name="big", bufs=1))
            xin = stack.enter_context(tc.tile_pool(name="xin", bufs=3))
            ev = stack.enter_context(tc.tile_pool(name="ev", bufs=3))
            # ---- constants (load order = first-use order) ----
            wq_sb = const.tile([128, NCC, C], bf16)
            wk_sb = const.tile([128, NCC, C], bf16)
            wv_sb = const.tile([128, NCC, C], bf16)
            wo_sb = const.tile([128, NCC, C], bf16)
            bq_sb = const.tile([128, NCC], f32)
            bo_sb = const.tile([128, NCC], f32)
            bkb_sb = const.tile([128, C], f32)
            bvb_sb = const.tile([128, C], f32)
            # Startup loads split across both HWDGE queues (SP + ACT): the
            # K-projection path (x j=0 interleaved with wk, then bkb) on SP,
            # everything else on the ACT queue.  The first kp matmul only
            # needs (x cc=0, wk cc=0), so interleaving starts PE ~1.3us in.
            # Startup loads run on two queues in parallel: x chunks on SP,
            # every weight on the otherwise-idle GpSimd (Pool) queue.  The
            # ACT queue stays free for the activation stream from t=0.
            # j=0 runs both K/V halves before any Q work, so wq only needs
            # to land ~7us in.
            xt0 = xin.tile([128, NCC, 512], bf16, name="xt0", tag="xt")
            for cc in range(NCC):
                nc.sync.dma_start(out=xt0[:, cc, :], in_=x_ap[:, cc, 0:512])
                nc.gpsimd.dma_start(out=wk_sb[:, cc, :],
                                    in_=wk_d.ap()[128 * cc:128 * (cc + 1), :])
            nc.sync.dma_start(out=bkb_sb, in_=bkb_d.ap())
            for cc in range(NCC):
                nc.gpsimd.dma_start(out=wv_sb[:, cc, :],
                                    in_=wv_d.ap()[128 * cc:128 * (cc + 1), :])
            nc.gpsimd.dma_start(out=bvb_sb, in_=bvb_d.ap())
            nc.scalar.dma_start(out=bq_sb, in_=bq_d.ap())
            for cc in range(NCC):
                nc.scalar.dma_start(out=wq_sb[:, cc, :],
                                    in_=wq_d.ap()[128 * cc:128 * (cc + 1), :])
            for cc in range(NCC):
                nc.gpsimd.dma_start(out=wo_sb[:, cc, :],
                                    in_=wo_d.ap()[128 * cc:128 * (cc + 1), :])
            nc.gpsimd.dma_start(out=bo_sb, in_=bo_d.ap())
            ones128_sb = const.tile([128, 64], bf16)
            nc.vector.memset(ones128_sb, 1.0)
            eye_sb = const.tile([128, 128], bf16)
            nc.gpsimd.dma_start(out=eye_sb, in_=eye_d.ap())

            # ---- persistent activations ----
            Q_sb = big.tile([128, NCC, L], bf16)     # [c, l] normal
            Kt_sb = big.tile([128, NL128, C], bf16)  # [l, c] transposed
            # V^T stored per m-chunk with a trailing ones column: [l, m, 129]
            # so the KV matmul's 129th output column accumulates Ksum in the
            # SAME psum chain (start=True zeroes a whole 2KB bank, so each
            # chain needs a private bank and no sibling chains).
            Vt_sb = big.tile([128, NL128, NCC, 129], bf16)
            nc.vector.memset(Vt_sb[:, :, :, 128:129], 1.0)
            # W~ = Wo @ blockdiag(KV_h^T) folded at the phase boundary: the
            # attention-out matmul then fuses into the O-projection (y =
            # W~ @ (Q*Z) + bo), dropping 32 out-matmuls from phase 2.
            KVbdT_sb = const.tile([128, NCC, 128], bf16)
            WtT_sb = const.tile([128, NCC, C], bf16)
            KVbd_sb = const.tile([128, NCC, 128], bf16)
            KsumRep_sb = const.tile([128, NCC, 128], bf16)
            ksum_sb = const.tile([128, NCC], f32)
            # zero the off-diagonal blocks up front (not data dependent)
            for m in range(NCC):
                nc.vector.memset(KVbd_sb[0:64, m, 64:128], 0.0)
                nc.vector.memset(KVbd_sb[64:128, m, 0:64], 0.0)
                nc.vector.memset(KsumRep_sb[0:64, m, 64:128], 0.0)
                nc.vector.memset(KsumRep_sb[64:128, m, 0:64], 0.0)

            def bcast_pair(ap):
                """View an AP ([128, ...]) with a broadcast pair dim inserted
                after partitions: [128, 2(step 0), ...]."""
                return bass.AP(tensor=ap.tensor, offset=ap.offset,
                               ap=[list(ap.ap[0]), [0, 2],
                                   *[list(d) for d in ap.ap[1:]]])

            # ================= phase 1: projections + KV accumulation =========
            with ExitStack() as p1stack:
                # PSUM (8 banks): kp x2 + vp x2 (4) + qp pair (2)
                #               + KV accumulators paired 2-per-bank (2)
                pkv = p1stack.enter_context(tc.tile_pool(name="pkv", bufs=2, space="PSUM"))
                pq = p1stack.enter_context(tc.tile_pool(name="pq", bufs=1, space="PSUM"))
                pacc = p1stack.enter_context(tc.tile_pool(name="pacc", bufs=1, space="PSUM"))
                # Two chains genuinely share one bank (chain g at 1KB offset
                # g*256 f32): only the temporally-first matmul in a bank
                # carries start=True -- start clears the has_written bits of
                # the WHOLE 2KB bank, so the second chain's first matmul must
                # NOT carry start or it would wipe its bank-sibling's li=0
                # contribution.  [128,129] at offsets 0/1024/2048/3072 never
                # crosses a bank boundary, keeping each matmul single-bank.
                KV_all = pacc.tile([128, 4, 256], f32, tag="kvacc", name="kv_all")
                KV_ps = [KV_all[:, g, 0:129] for g in range(4)]

                bvb4 = bvb_sb[:, :].rearrange("p (m c) -> p m c", m=NCC)
                bkb2 = bcast_pair(bkb_sb[:, :])
                bvb4x2 = bcast_pair(bvb4)
                # Q's final combine is deferred one half-iteration: it is only
                # consumed in phase 2, and emitting it late keeps the DVE FIFO
                # clear so the next half's psum-releasing adds run sooner.
                pending_qstt = []
                # KV-accumulation matmuls are likewise deferred one half:
                # they depend on the elementwise K/V chain (~4-5us deep) and
                # the PE instruction FIFO is strict in-order, so emitting them
                # immediately would head-of-line-block the next half's
                # independent projection matmuls.
                pending_kv = []

                def emit_kv(li0):
                    for jj2 in range(2):
                        li = li0 + jj2
                        for m in range(NCC):
                            ks = Kt_sb[:, li, 128 * m:128 * (m + 1)]
                            nc.tensor.matmul(
                                KV_ps[m], ks, Vt_sb[:, li, m, :],
                                start=(li == 0 and m % 2 == 0),
                                stop=(li == NL128 - 1))
                for j in range(NL512):
                    if j == 0:
                        xt = xt0
                    else:
                        xt = xin.tile([128, NCC, 512], bf16, name="xt", tag="xt")
                        nc.sync.dma_start(out=xt,
                                          in_=x_ap[:, :, 512 * j:512 * (j + 1)])

                    def kv_half(j, half, xt):
                        # Both 128-l chunks of this half processed as one
                        # [128, 2, 512] pair: halves the per-op fixed cost on
                        # every DVE/ACT instruction in the K/V chain.
                        li0 = 4 * j + 2 * half
                        kp = pkv.tile([128, 2, 512], f32, tag="kp", name="kp", bufs=1)
                        vp = pkv.tile([128, 2, 512], f32, tag="vp", name="vp", bufs=1)
                        for jj2 in range(2):
                            lf = 128 * (2 * half + jj2)
                            for cc in range(NCC):
                                xs = xt[:, cc, lf:lf + 128]
                                nc.tensor.matmul(kp[:, jj2, :], xs, wk_sb[:, cc, :],
                                                 start=(cc == 0), stop=(cc == NCC - 1))
                        for jj2 in range(2):
                            lf = 128 * (2 * half + jj2)
                            for cc in range(NCC):
                                xs = xt[:, cc, lf:lf + 128]
                                nc.tensor.matmul(vp[:, jj2, :], xs, wv_sb[:, cc, :],
                                                 start=(cc == 0), stop=(cc == NCC - 1))
                        # K^T = elu(kp + bk) + 1 = min(exp(t),1) + relu(t)
                        t0 = ev.tile([128, 2, 512], f32, tag="t0", bufs=3, name="t0")
                        e0 = ev.tile([128, 2, 512], bf16, tag="e0", bufs=4, name="e0")
                        r0 = ev.tile([128, 2, 512], bf16, tag="r0", bufs=4, name="r0")
                        nc.vector.tensor_add(t0, kp, bkb2)
                        # V^T = vp + bv  (written into the 129-strided layout)
                        nc.vector.tensor_add(
                            Vt_sb[:, li0:li0 + 2, :, 0:128],
                            vp.rearrange("p two (m c) -> p two m c", m=NCC),
                            bvb4x2)
                        nc.scalar.activation(e0, t0, AF.Exp)
                        nc.scalar.activation(r0, t0, AF.Relu)
                        nc.vector.scalar_tensor_tensor(
                            Kt_sb[:, li0:li0 + 2, :], e0, 1.0, r0, OP.min, OP.add)
                        pending_kv.append(li0)

                    def q_proj(j, op2, xt):
                        # Q projection, one oi-pair (spreads ACT load).  Two
                        # single-bank psum tags rotate so oi+1's matmuls can
                        # run while oi's ACT evictions still read their bank.
                        eq = ev.tile([128, 2, 512], bf16, tag="e0", bufs=4, name="eq")
                        rq = ev.tile([128, 2, 512], bf16, tag="r0", bufs=4, name="rq")
                        for oi2 in range(2):
                            oi = 2 * op2 + oi2
                            qp = pq.tile([128, 512], f32, tag=f"qp{oi2}",
                                         name=f"qp{oi2}")
                            for cc in range(NCC):
                                nc.tensor.matmul(
                                    qp,
                                    wq_sb[:, cc, 128 * oi:128 * (oi + 1)],
                                    xt[:, cc, :],
                                    start=(cc == 0), stop=(cc == NCC - 1))
                            nc.scalar.activation(eq[:, oi2, :], qp,
                                                 AF.Exp, bias=bq_sb[:, oi:oi + 1])
                            nc.scalar.activation(rq[:, oi2, :], qp,
                                                 AF.Relu, bias=bq_sb[:, oi:oi + 1])
                            if pending_qstt:
                                nc.vector.scalar_tensor_tensor(*pending_qstt.pop())
                            pending_qstt.append(
                                (Q_sb[:, oi, 512 * j:512 * (j + 1)],
                                 eq[:, oi2, :], 1.0, rq[:, oi2, :],
                                 OP.min, OP.add))

                    if j == 0:
                        # first chunk: both K/V halves before any Q work, so
                        # the PE never waits on the later-arriving wq and the
                        # ACT queue warms up on the K-path exp/relu stream
                        kv_half(j, 0, xt)
                        kv_half(j, 1, xt)
                        emit_kv(pending_kv.pop(0))
                        q_proj(j, 0, xt)
                        q_proj(j, 1, xt)
                    elif j < NL512 - 1:
                        for half in range(2):
                            kv_half(j, half, xt)
                            if len(pending_kv) > 1:
                                emit_kv(pending_kv.pop(0))
                            q_proj(j, half, xt)
                    else:
                        # last chunk: K/V first so the final elementwise chain
                        # overlaps the Q projections, keeping the dangling
                        # stt -> KV-matmul tail at the phase boundary short
                        kv_half(j, 0, xt)
                        emit_kv(pending_kv.pop(0))
                        kv_half(j, 1, xt)
                        q_proj(j, 0, xt)
                        emit_kv(pending_kv.pop(0))
                        q_proj(j, 1, xt)
                while pending_kv:
                    emit_kv(pending_kv.pop(0))

                # ---- evict Ksum + KsumRep on DVE (S(j=0) depends only on
                # these); the KVbd blocks evict on ACT (idle at the
                # boundary) so both chains run in parallel ----
                for m in range(NCC):
                    nc.vector.tensor_copy(ksum_sb[:, m:m + 1],
                                          KV_ps[m][:, 128:129])
                for m in range(NCC):
                    nc.vector.tensor_scalar_mul(
                        KsumRep_sb[0:64, m, 0:64], ones128_sb[0:64, :],
                        ksum_sb[0:64, m:m + 1])
                    nc.vector.tensor_scalar_mul(
                        KsumRep_sb[64:128, m, 64:128], ones128_sb[64:128, :],
                        ksum_sb[64:128, m:m + 1])
                for m in range(NCC):
                    kv_m = KV_ps[m]
                    nc.scalar.copy(KVbd_sb[0:64, m, 0:64], kv_m[0:64, 0:64])
                    nc.scalar.copy(KVbd_sb[64:128, m, 64:128],
                                   kv_m[64:128, 64:128])
                # the last Q combine is only consumed by phase-2 j=7 -- flush
                # it AFTER the KV eviction chain so it doesn't delay S_b(j=0)
                if pending_qstt:
                    nc.vector.scalar_tensor_tensor(*pending_qstt.pop())

            if debug_outputs:
                nc.sync.dma_start(out=dbg["Q"].ap(), in_=Q_sb)
                nc.sync.dma_start(out=dbg["Kt"].ap(), in_=Kt_sb)
                nc.sync.dma_start(out=dbg["Vt"].ap(), in_=Vt_sb)
                nc.sync.dma_start(out=dbg["KVbd"].ap(), in_=KVbd_sb)
                nc.sync.dma_start(out=dbg["KsumRep"].ap(), in_=KsumRep_sb)

            # ================= phase 2: fold + O-projection ===================
            # W~T[m] = KVbd[m]^T @ WoT[m] is built once at the boundary; the
            # j-loop then only needs S (KsumRep matmuls) -> 1/S -> scale Q in
            # place -> y = W~T^T @ Q~.  Z rides on Q, so the old per-j
            # attention-out matmuls disappear entirely.
            with ExitStack() as p2stack:
                # PSUM banks: sb pair (2) + y pair x2 bufs (4) + tp (1);
                # the wt fold tiles share the y0 tag's banks.
                p2 = p2stack.enter_context(tc.tile_pool(name="p2", bufs=1, space="PSUM"))
                p2o = p2stack.enter_context(tc.tile_pool(name="p2o", bufs=2, space="PSUM"))
                # ytp deep enough that output-DMA latency never back-
                # pressures the ACT evictions (and with them the y matmuls)
                ytp = p2stack.enter_context(tc.tile_pool(name="ytp", bufs=4))

                def s_group(j):
                    lsl = slice(512 * j, 512 * (j + 1))
                    for mp in range(2):
                        sbp = p2.tile([128, 2, 512], f32, tag="sb", name="sbp",
                                      bufs=2)
                        for m2 in range(2):
                            m = 2 * mp + m2
                            nc.tensor.matmul(sbp[:, m2, :], KsumRep_sb[:, m, :],
                                             Q_sb[:, m, lsl], start=True, stop=True)
                        # Q~ = Q/S in place via the fused approx-recip+mul op
                        # (S above already read this slice)
                        qsl = Q_sb[:, 2 * mp:2 * mp + 2, lsl]
                        nc.vector._custom_dve(
                            recip1_mul, out=qsl, in0=sbp, in1=qsl,
                            s0=_RECIP1_MUL_CONSTS[0], s1=_RECIP1_MUL_CONSTS[1],
                            imm2=0.0)

                def fold_wt():
                    # Per-m pipelined fold: transpose chunk m via eye, copy
                    # it out, multiply by WoT[m], evict -- so W~T[m=0] (the
                    # first lhsT the y matmuls need) is ready ~2us after the
                    # KVbd eviction instead of after the whole fold.  tp
                    # borrows an "sb" slot (only live across the boundary).
                    tp = p2.tile([128, NCC, 128], f32, tag="sb", name="tp",
                                 bufs=2)
                    for m in range(NCC):
                        nc.tensor.matmul(tp[:, m, :], KVbd_sb[:, m, :], eye_sb,
                                         start=(m == 0), stop=(m == NCC - 1))
                        nc.scalar.copy(KVbdT_sb[:, m, :], tp[:, m, :])
                        wt = p2o.tile([128, 512], f32, tag="y0", name="wt", bufs=2)
                        nc.tensor.matmul(wt, KVbdT_sb[:, m, :], wo_sb[:, m, :],
                                         start=True, stop=True)
                        if m % 2 == 0:
                            nc.scalar.activation(WtT_sb[:, m, :], wt, AF.Identity)
                        else:
                            nc.vector.tensor_copy(WtT_sb[:, m, :], wt)

                # output DMAs rotate across four queues so the write-back
                # never serializes behind one HWDGE queue; fine mode drains
                # per-oi as soon as each chunk is evicted.
                dma_qs = [nc.sync, nc.gpsimd, nc.scalar, nc.gpsimd]
                def y_block(j, fine=False):
                    lsl = slice(512 * j, 512 * (j + 1))
                    for op2 in range(2):
                        dma_q = dma_qs[(2 * j + op2) % len(dma_qs)]
                        yt = ytp.tile([128, 2, 512], f32, name="yt")
                        for oi2 in range(2):
                            oi = 2 * op2 + oi2
                            yp = p2o.tile([128, 512], f32, tag=f"y{oi2}",
                                          name="yp", bufs=2)
                            for vi in range(NCC):
                                nc.tensor.matmul(
                                    yp,
                                    WtT_sb[:, vi, 128 * oi:128 * (oi + 1)],
                                    Q_sb[:, vi, lsl],
                                    start=(vi == 0), stop=(vi == NCC - 1))
                            nc.scalar.activation(yt[:, oi2, :], yp,
                                                 AF.Identity, bias=bo_sb[:, oi:oi + 1])
                            if fine:  # drain the tail with per-oi DMAs
                                dma_qs[oi % 3].dma_start(
                                    out=out_ap[128 * oi:128 * (oi + 1), lsl],
                                    in_=yt[:, oi2, :])
                        if not fine:
                            dma_q.dma_start(
                                out=out_ap[256 * op2:256 * (op2 + 1), lsl].rearrange(
                                    "(two p) l -> p two l", p=128),
                                in_=yt)

                # Emission order at the boundary untangles the per-engine
                # FIFOs: tp needs only the KVbd eviction, s_group(0) needs
                # only KsumRep, and putting s_group(0)'s DVE work after the
                # KVbdT copy keeps fold_wt's matmuls from waiting behind the
                # j=0 recip chain.  y-blocks run one j behind so their
                # matmuls fill the PE while the next j's S/scale chain is in
                # flight.
                s_group(0)
                fold_wt()
                for j in range(1, NL512):
                    s_group(j)
                    y_block(j - 1, fine=(j == NL512 - 1))
                y_block(NL512 - 1, fine=True)

    nc.compile()
    return nc


def _get_nc():
    if "nc" not in _CACHE:
        _CACHE["nc"] = _build_nc()
    return _CACHE["nc"]


def _make_in_maps(inputs):
    x = np.asarray(inputs["x"], dtype=np.float32)
    wq = np.asarray(inputs["wq"], dtype=np.float32)
    wk = np.asarray(inputs["wk"], dtype=np.float32)
    wv = np.asarray(inputs["wv"], dtype=np.float32)
    wo = np.asarray(inputs["wo"], dtype=np.float32)
    bq = np.asarray(inputs["bq"], dtype=np.float32)
    bk = np.asarray(inputs["bk"], dtype=np.float32)
    bv = np.asarray(inputs["bv"], dtype=np.float32)
    bo = np.asarray(inputs["bo"], dtype=np.float32)

    shared = {
        "wqT": np.ascontiguousarray(wq.T).astype(BF16),
        "wkT": np.ascontiguousarray(wk.T).astype(BF16),
        "wvT": np.ascontiguousarray(wv.T).astype(BF16),
        "woT": np.ascontiguousarray(wo.T).astype(BF16),
        "bqT": np.ascontiguousarray(bq.reshape(NCC, 128).T),
        "boT": np.ascontiguousarray(bo.reshape(NCC, 128).T),
        "bkb": np.ascontiguousarray(np.broadcast_to(bk, (128, C))),
        "bvb": np.ascontiguousarray(np.broadcast_to(bv, (128, C))),
        "eye": np.eye(128, dtype=np.float32).astype(BF16),
    }
    in_maps = []
    for b in range(NB):
        m = dict(shared)
        m["x"] = np.ascontiguousarray(x[b].reshape(C, L)).astype(BF16)
        in_maps.append(m)
    return in_maps


def _run(inputs, trace=False):
    from concourse.bass_utils import run_bass_kernel_spmd

    nc = _get_nc()
    in_maps = _make_in_maps(inputs)
    res = run_bass_kernel_spmd(nc, in_maps, core_ids=list(range(NB)), trace=trace)
    outs = np.stack([np.asarray(res.results[b]["out"], dtype=np.float32)
                     for b in range(NB)])
    y = outs.reshape(NB, C, 64, 64)
    return y, res


def kernel(**inputs) -> np.ndarray:
    y, _ = _run(inputs, trace=False)
    return y

